# revision 30
# baseline (speedup 1.0000x reference)
"""Trainium2 Bass kernel for the scalar-gain Kalman filter.

Math: the reference recurrence x_k = x_{k-1} + K_k (z_k - x_{k-1}) has
data-independent scalar gains K_k (they depend only on log_Q/log_R), so
the whole filter is a linear map along the time axis:

    x[n, k] = sum_j L[k, j] * z[n, j],   L[k, j] = K_j * prod_{i=j+1..k} (1 - K_i)

with K_0 := 1.  L is lower-triangular 512x512, computed on the host from
the two scalar params.  Because |1-K_i| converges to ~0.382, L[k, j]
decays geometrically in (k-j); entries with k-j >= 128 are < 1e-53, so
restricting L to a 2-block band (current + previous 128-wide time chunk)
is exact at f32 precision.

Implementation (v3):
  - The 2e-2 rel-err budget admits bf16 end to end: the host casts z to
    bf16 AND pre-transposes each core's shard to z^T [512, 8192], so the
    device needs no PE transposes at all and input HBM traffic halves.
    The device output is bf16 too (PSUM accumulation stays fp32; one
    rounding on the PSUM->SBUF copy), halving output traffic.  Total
    HBM traffic per core: 8.4 MB in + 8.4 MB out ~= 47 us at the
    358 GB/s per-core HBM limit, vs ~95 us for fp32 I/O.
  - Per output row-tile [128 rows, 512 times]: 4 bf16 matmuls
    (stationary = z^T chunk [128 j, 128 rows] sliced from a resident
    SBUF tile, moving = banded L^T [diag_q | prev_{q+1}], N=256) into
    one PSUM bank; ACT/DVE copy PSUM->SBUF casting fp32->bf16; merged
    4-tile output DMA via a rearranged DRAM access pattern (fewer, (
    bigger DMA issues).
  - Input: 16 DMAs of [128, 2048] bf16 (4 KB/partition contiguous),
    issued up front; all of z^T (8.4 MB) stays resident in SBUF.
"""

import ml_dtypes
import numpy as np

import concourse.bass as bass
import concourse.mybir as mybir
from concourse import bacc
from concourse import bass_utils
from concourse.tile import TileContext

B, C, W = 64, 1024, 512
NCORES = 8
ROWS = B * C // NCORES  # 8192 rows per core
P = 128                 # partitions / row-tile height
NT = ROWS // P          # 64 row-tiles per core
CH = 128                # time chunk
NCH = W // CH           # 4 chunks
# Matmul schedule per output tile: (j_chunk, kolumn_off, ncols, start, stop).
# PSUM start=True zeroes at 2KB (whole-bank) granularity, so exactly one
# start=True matmul must cover all 512 columns (its band plus explicit
# zeros); the rest accumulate into the fully-written bank.
_MMS = [
    (0, 0, W, True, False),        # [diag_0 | prev_1 | zeros] -> cols 0..511
    (1, CH, 2 * CH, False, False),  # [diag_1 | prev_2] -> cols 128..383
    (2, 2 * CH, 2 * CH, False, False),  # [diag_2 | prev_3] -> cols 256..511
    (3, 3 * CH, CH, False, True),  # [diag_3]          -> cols 384..511
]
_LT_OFFS = [0]
for _mm in _MMS[:-1]:
    _LT_OFFS.append(_LT_OFFS[-1] + _mm[2])
LTW = _LT_OFFS[-1] + _MMS[-1][2]  # 896 packed L^T columns
RB = 1024               # input row-block (rows per input DMA tile)
NRB = ROWS // RB        # 8 row-blocks
TPR = RB // P           # 8 output tiles per row-block
MT = 4                  # output tiles merged per output DMA

_cache = {}


def _build_nc():
    nc = bacc.Bacc(
        "TRN2",
        target_bir_lowering=False,
        debug=False,
        enable_asserts=False,
        num_devices=NCORES,
    )
    zt = nc.dram_tensor("zt", [W, ROWS], mybir.dt.bfloat16, kind="ExternalInput").ap()
    lt = nc.dram_tensor("lt", [P, LTW], mybir.dt.bfloat16, kind="ExternalInput").ap()
    out = nc.dram_tensor("out", [ROWS, W], mybir.dt.bfloat16, kind="ExternalOutput").ap()
    # view for merged MT-tile output DMAs: [group, partition, tile, col]
    out_g = out.rearrange("(g k p) c -> g p k c", k=MT, p=P)

    with TileContext(nc) as tc:
        with (
            tc.tile_pool(name="const", bufs=1) as constp,
            tc.tile_pool(name="ztin", bufs=NRB * NCH) as ztinp,
            tc.tile_pool(name="res", bufs=8) as resp,
            tc.tile_pool(name="outps", bufs=8, space="PSUM") as outpsp,
        ):
            ltt = constp.tile([P, LTW], mybir.dt.bfloat16)
            nc.sync.dma_start(ltt[:], lt)

            # resident z^T: NRB*4 tiles of [128 j, RB rows], 2 KB/partition.
            # All DMA issues (in and out) go on Sync.  DMA_DIRECT2D issues
            # chain on 8 completion-semaphore lanes, so issuing all 32
            # inputs up front would pin Sync for ~20 us and delay the
            # output issues queued behind them; instead prefetch 2 row-
            # blocks ahead and interleave the rest with the tile loop.
            zts = [[None] * NCH for _ in range(NRB)]
            pending = [(rb, q) for rb in range(NRB) for q in range(NCH)]

            def _issue_in(n):
                for rb, q in pending[:n]:
                    zin = ztinp.tile([P, RB], mybir.dt.bfloat16)
                    nc.sync.dma_start(
                        zin[:], zt[q * CH : (q + 1) * CH, rb * RB : (rb + 1) * RB]
                    )
                    zts[rb][q] = zin
                del pending[:n]

            # 3 row-blocks up front, then trickle 2 issues per even tile so
            # input stays ~3 blocks ahead without pinning the Sync queue
            # (output DMA issues interleave between them).
            _issue_in(3 * NCH)

            res = None
            for t in range(NT):
                rb, tt = divmod(t, TPR)
                g, s = divmod(t, MT)
                if t % 2 == 0 and pending:
                    _issue_in(2)
                ops = outpsp.tile([P, W], mybir.dt.float32)
                for mi, (j, off, ncols, mstart, mstop) in enumerate(_MMS):
                    nc.tensor.matmul(
                        ops[:, off : off + ncols],
                        zts[rb][j][:, tt * P : (tt + 1) * P],
                        ltt[:, _LT_OFFS[mi] : _LT_OFFS[mi] + ncols],
                        start=mstart,
                        stop=mstop,
                        skip_group_check=True,
                    )

                if s == 0:
                    res = resp.tile([P, MT * W], mybir.dt.bfloat16)
                # PSUM->SBUF copy (casts fp32->bf16), alternating DVE/ACT.
                if t % 2 == 0:
                    nc.vector.tensor_copy(res[:, s * W : (s + 1) * W], ops[:])
                else:
                    nc.scalar.copy(res[:, s * W : (s + 1) * W], ops[:])
                if s == MT - 1:
                    # g odd: the group's last copy (t odd) just ran on
                    # Scalar, so its out-issue on Scalar starts stall-free
                    # and halves the Sync issue stream.
                    oeng = nc.sync if g % 2 == 0 else nc.scalar
                    oeng.dma_start(out_g[g], res[:])
    nc.compile()
    return nc


def _gains(log_Q, log_R):
    """Replicate the reference f32 scalar scan for the Kalman gains."""
    f32 = np.float32
    Q = f32(np.exp(f32(log_Q)))
    R = f32(np.exp(f32(log_R)))
    Pv = f32(Q + R)
    Ks = np.empty(W, np.float64)
    Ks[0] = 1.0  # x_0 = z_0
    for k in range(1, W):
        P_pred = f32(Pv + Q)
        K = f32(P_pred / f32(P_pred + R))
        Pv = f32(f32(1.0 - K) * P_pred)
        Ks[k] = K
    return Ks


def _lt_pack(log_Q, log_R):
    """Banded spans of L^T, packed [128, LTW] bf16.

    Span i is L[koff:koff+ncols, jc]^T for (jc, koff, ncols) = _MMS[i],
    with partition = j (the contraction dim), free = k.  Entries outside
    the band (k < j or k - j >= 256) are exactly zero.
    """
    Ks = _gains(log_Q, log_R)
    a = 1.0 - Ks
    a[0] = 1.0
    cp = np.cumprod(a)  # cp[k] = prod_{i<=k} a_i  (a_0 = 1)
    # L[k, j] = Ks[j] * cp[k] / cp[j]  for j <= k
    k_idx = np.arange(W)
    Lf = Ks[None, :] * (cp[:, None] / cp[None, :])
    Lf = np.where(k_idx[None, :] <= k_idx[:, None], Lf, 0.0)
    # band limit: contributions with k - j >= 256 are < 1e-100, drop them
    Lf = np.where(k_idx[:, None] - k_idx[None, :] < 2 * CH, Lf, 0.0)

    blocks = []
    for j, koff, ncols, _, _ in _MMS:
        js = slice(j * CH, (j + 1) * CH)
        blocks.append(Lf[koff : koff + ncols, js].T)
    return np.ascontiguousarray(
        np.concatenate(blocks, axis=1).astype(ml_dtypes.bfloat16)
    )


def _get_nc():
    nc = _cache.get("nc")
    if nc is None:
        nc = _build_nc()
        _cache["nc"] = nc
    return nc


def run_sharded(z, log_Q, log_R, **spmd_kwargs):
    """Run the SPMD kernel; returns (full_output, BassKernelResults)."""
    nc = _get_nc()
    ltp = _lt_pack(np.asarray(log_Q).reshape(-1)[0], np.asarray(log_R).reshape(-1)[0])
    zb = np.asarray(z, np.float32).reshape(NCORES, ROWS, W).astype(ml_dtypes.bfloat16)
    in_maps = [
        {"zt": np.ascontiguousarray(zb[i].T), "lt": ltp} for i in range(NCORES)
    ]
    res = bass_utils.run_bass_kernel_spmd(
        nc, in_maps, core_ids=list(range(NCORES)), **spmd_kwargs
    )
    full = (
        np.concatenate([r["out"] for r in res.results], axis=0)
        .reshape(B, C, W)
        .astype(np.float32)
    )
    return full, res


def kernel(z, log_Q, log_R):
    full, _ = run_sharded(z, log_Q, log_R)
    return full


# revision 33
# speedup vs baseline: 1.0027x; 1.0027x over previous
"""Trainium2 Bass kernel for the scalar-gain Kalman filter.

Math: the reference recurrence x_k = x_{k-1} + K_k (z_k - x_{k-1}) has
data-independent scalar gains K_k (they depend only on log_Q/log_R), so
the whole filter is a linear map along the time axis:

    x[n, k] = sum_j L[k, j] * z[n, j],   L[k, j] = K_j * prod_{i=j+1..k} (1 - K_i)

with K_0 := 1.  L is lower-triangular 512x512, computed on the host from
the two scalar params.  Because |1-K_i| converges to ~0.382, L[k, j]
decays geometrically in (k-j); entries with k-j >= 128 are < 1e-53, so
restricting L to a 2-block band (current + previous 128-wide time chunk)
is exact at f32 precision.

Implementation (v3):
  - The 2e-2 rel-err budget admits bf16 end to end: the host casts z to
    bf16 AND pre-transposes each core's shard to z^T [512, 8192], so the
    device needs no PE transposes at all and input HBM traffic halves.
    The device output is bf16 too (PSUM accumulation stays fp32; one
    rounding on the PSUM->SBUF copy), halving output traffic.  Total
    HBM traffic per core: 8.4 MB in + 8.4 MB out ~= 47 us at the
    358 GB/s per-core HBM limit, vs ~95 us for fp32 I/O.
  - Per output row-tile [128 rows, 512 times]: 4 bf16 matmuls
    (stationary = z^T chunk [128 j, 128 rows] sliced from a resident
    SBUF tile, moving = banded L^T [diag_q | prev_{q+1}], N=256) into
    one PSUM bank; ACT/DVE copy PSUM->SBUF casting fp32->bf16; merged
    4-tile output DMA via a rearranged DRAM access pattern (fewer, (
    bigger DMA issues).
  - Input: 16 DMAs of [128, 2048] bf16 (4 KB/partition contiguous),
    issued up front; all of z^T (8.4 MB) stays resident in SBUF.
"""

import ml_dtypes
import numpy as np

import concourse.bass as bass
import concourse.mybir as mybir
from concourse import bacc
from concourse import bass_utils
from concourse.tile import TileContext

B, C, W = 64, 1024, 512
NCORES = 8
ROWS = B * C // NCORES  # 8192 rows per core
P = 128                 # partitions / row-tile height
NT = ROWS // P          # 64 row-tiles per core
CH = 128                # time chunk
NCH = W // CH           # 4 chunks
# Matmul schedule per output tile: (j_chunk, kolumn_off, ncols, start, stop).
# PSUM start=True zeroes at 2KB (whole-bank) granularity, so exactly one
# start=True matmul must cover all 512 columns (its band plus explicit
# zeros); the rest accumulate into the fully-written bank.
_MMS = [
    (0, 0, W, True, False),        # [diag_0 | prev_1 | zeros] -> cols 0..511
    (1, CH, 2 * CH, False, False),  # [diag_1 | prev_2] -> cols 128..383
    (2, 2 * CH, 2 * CH, False, False),  # [diag_2 | prev_3] -> cols 256..511
    (3, 3 * CH, CH, False, True),  # [diag_3]          -> cols 384..511
]
_LT_OFFS = [0]
for _mm in _MMS[:-1]:
    _LT_OFFS.append(_LT_OFFS[-1] + _mm[2])
LTW = _LT_OFFS[-1] + _MMS[-1][2]  # 896 packed L^T columns
RB = 1024               # input row-block (rows per input DMA tile)
NRB = ROWS // RB        # 8 row-blocks
TPR = RB // P           # 8 output tiles per row-block
MT = 8                  # output tiles merged per output DMA

_cache = {}


def _build_nc():
    nc = bacc.Bacc(
        "TRN2",
        target_bir_lowering=False,
        debug=False,
        enable_asserts=False,
        num_devices=NCORES,
    )
    zt = nc.dram_tensor("zt", [W, ROWS], mybir.dt.bfloat16, kind="ExternalInput").ap()
    lt = nc.dram_tensor("lt", [P, LTW], mybir.dt.bfloat16, kind="ExternalInput").ap()
    out = nc.dram_tensor("out", [ROWS, W], mybir.dt.bfloat16, kind="ExternalOutput").ap()
    # view for merged MT-tile output DMAs: [group, partition, tile, col]
    out_g = out.rearrange("(g k p) c -> g p k c", k=MT, p=P)

    with TileContext(nc) as tc:
        with (
            tc.tile_pool(name="const", bufs=1) as constp,
            tc.tile_pool(name="ztin", bufs=NRB * NCH) as ztinp,
            tc.tile_pool(name="res", bufs=4) as resp,
            tc.tile_pool(name="outps", bufs=8, space="PSUM") as outpsp,
        ):
            ltt = constp.tile([P, LTW], mybir.dt.bfloat16)
            nc.sync.dma_start(ltt[:], lt)

            # resident z^T: NRB*4 tiles of [128 j, RB rows], 2 KB/partition.
            # All DMA issues (in and out) go on Sync.  DMA_DIRECT2D issues
            # chain on 8 completion-semaphore lanes, so issuing all 32
            # inputs up front would pin Sync for ~20 us and delay the
            # output issues queued behind them; instead prefetch 2 row-
            # blocks ahead and interleave the rest with the tile loop.
            zts = [[None] * NCH for _ in range(NRB)]
            pending = [(rb, q) for rb in range(NRB) for q in range(NCH)]

            def _issue_in(n):
                for rb, q in pending[:n]:
                    zin = ztinp.tile([P, RB], mybir.dt.bfloat16)
                    nc.sync.dma_start(
                        zin[:], zt[q * CH : (q + 1) * CH, rb * RB : (rb + 1) * RB]
                    )
                    zts[rb][q] = zin
                del pending[:n]

            # 3 row-blocks up front, then trickle 2 issues per even tile so
            # input stays ~3 blocks ahead without pinning the Sync queue
            # (output DMA issues interleave between them).
            _issue_in(3 * NCH)

            res = None
            for t in range(NT):
                rb, tt = divmod(t, TPR)
                g, s = divmod(t, MT)
                if t % 2 == 0 and pending:
                    _issue_in(2)
                ops = outpsp.tile([P, W], mybir.dt.float32)
                for mi, (j, off, ncols, mstart, mstop) in enumerate(_MMS):
                    nc.tensor.matmul(
                        ops[:, off : off + ncols],
                        zts[rb][j][:, tt * P : (tt + 1) * P],
                        ltt[:, _LT_OFFS[mi] : _LT_OFFS[mi] + ncols],
                        start=mstart,
                        stop=mstop,
                        skip_group_check=True,
                    )

                if s == 0:
                    res = resp.tile([P, MT * W], mybir.dt.bfloat16)
                # PSUM->SBUF copy (casts fp32->bf16), alternating DVE/ACT.
                if t % 2 == 0:
                    nc.vector.tensor_copy(res[:, s * W : (s + 1) * W], ops[:])
                else:
                    nc.scalar.copy(res[:, s * W : (s + 1) * W], ops[:])
                if s == MT - 1:
                    nc.sync.dma_start(out_g[g], res[:])
    nc.compile()
    return nc


def _gains(log_Q, log_R):
    """Replicate the reference f32 scalar scan for the Kalman gains."""
    f32 = np.float32
    Q = f32(np.exp(f32(log_Q)))
    R = f32(np.exp(f32(log_R)))
    Pv = f32(Q + R)
    Ks = np.empty(W, np.float64)
    Ks[0] = 1.0  # x_0 = z_0
    for k in range(1, W):
        P_pred = f32(Pv + Q)
        K = f32(P_pred / f32(P_pred + R))
        Pv = f32(f32(1.0 - K) * P_pred)
        Ks[k] = K
    return Ks


def _lt_pack(log_Q, log_R):
    """Banded spans of L^T, packed [128, LTW] bf16.

    Span i is L[koff:koff+ncols, jc]^T for (jc, koff, ncols) = _MMS[i],
    with partition = j (the contraction dim), free = k.  Entries outside
    the band (k < j or k - j >= 256) are exactly zero.
    """
    Ks = _gains(log_Q, log_R)
    a = 1.0 - Ks
    a[0] = 1.0
    cp = np.cumprod(a)  # cp[k] = prod_{i<=k} a_i  (a_0 = 1)
    # L[k, j] = Ks[j] * cp[k] / cp[j]  for j <= k
    k_idx = np.arange(W)
    Lf = Ks[None, :] * (cp[:, None] / cp[None, :])
    Lf = np.where(k_idx[None, :] <= k_idx[:, None], Lf, 0.0)
    # band limit: contributions with k - j >= 256 are < 1e-100, drop them
    Lf = np.where(k_idx[:, None] - k_idx[None, :] < 2 * CH, Lf, 0.0)

    blocks = []
    for j, koff, ncols, _, _ in _MMS:
        js = slice(j * CH, (j + 1) * CH)
        blocks.append(Lf[koff : koff + ncols, js].T)
    return np.ascontiguousarray(
        np.concatenate(blocks, axis=1).astype(ml_dtypes.bfloat16)
    )


def _get_nc():
    nc = _cache.get("nc")
    if nc is None:
        nc = _build_nc()
        _cache["nc"] = nc
    return nc


def run_sharded(z, log_Q, log_R, **spmd_kwargs):
    """Run the SPMD kernel; returns (full_output, BassKernelResults)."""
    nc = _get_nc()
    ltp = _lt_pack(np.asarray(log_Q).reshape(-1)[0], np.asarray(log_R).reshape(-1)[0])
    zb = np.asarray(z, np.float32).reshape(NCORES, ROWS, W).astype(ml_dtypes.bfloat16)
    in_maps = [
        {"zt": np.ascontiguousarray(zb[i].T), "lt": ltp} for i in range(NCORES)
    ]
    res = bass_utils.run_bass_kernel_spmd(
        nc, in_maps, core_ids=list(range(NCORES)), **spmd_kwargs
    )
    full = (
        np.concatenate([r["out"] for r in res.results], axis=0)
        .reshape(B, C, W)
        .astype(np.float32)
    )
    return full, res


def kernel(z, log_Q, log_R):
    full, _ = run_sharded(z, log_Q, log_R)
    return full
